# revision 26
# baseline (speedup 1.0000x reference)
"""CrossModalAttention kernel for 8x TRN2 NeuronCores (batch data-parallel).

Reference computation (per batch element b, context input is unused):
    qkv = x @ qkv_w + qkv_b            # [N, 3C]
    q, k, v = split(qkv)               # heads H=12, d=64
    attn = softmax(q*scale @ k^T)      # per head, N=1024
    out = (attn @ v) @ proj_w + proj_b # [N, C]

Strategy per core (one batch element each). v5 -- all-fp16 matmuls
(fp8/DoubleRow measured 7e-2 rel err: ~3% per-element quantization does
NOT average out in random-sign contractions), head-pair loop structure,
stall elimination:

  - Head-PAIR scores: head 2p lives in partitions 0:64 and head 2p+1 in
    64:128 of the qkT chunks, so the two scores matmuls (K=64) target
    disjoint PE row-groups via auto tile_position and partially overlap.
    One [128,1024] PSUM tile per (kc, q-half) = even|odd halves, one
    N=1024 exp covers both heads (minimizes ACT instruction count; ACT
    exp is the second-busiest engine at ~110us).
  - v_aug padded to 128 cols/head (64 v dims + ones + 63 zeros): full
    128-col stationary operand re-enables fast weight load; M=65 was
    measured +40%/matmul from serialized LDWEIGHTS. PSUM cost unchanged
    (rows 65:128 of the av banks were dead anyway).
  - Normalization: the whole av [65,1024] is copied PSUM->SBUF first,
    releasing the PSUM bank in ~0.8us instead of holding it through the
    reciprocal + DRAM-broadcast chain (~5us); this killed the ~5us
    per-pair-boundary ACT stalls seen in the v4 trace. Then 1/sums via
    DVE fast reciprocal, partition-broadcast via DRAM round trip, DVE
    multiply into fp16 outT.
  - qkv weight DMAs split so the first columns (m-chunks 0 and 6) land
    first: the first scores matmul unblocks ~4us earlier.
  - proj split 3 ways: A = c 0..2 (after pair 2), B = c 3..4 (+DVE add,
    during pair 5), finish = c5 after the last normalization.
  PSUM (8 banks): sc ring 2x[128,1024] = 4 (also carries qkv/v/proj
  filler groups), av_e+av_o 2x[128,1024] = 4.
"""
import numpy as np

import concourse.bass as bass
import concourse.tile as tile
from concourse import bacc, mybir
from concourse.bass_utils import run_bass_kernel_spmd

DIM = 768
NUM_HEADS = 12
HEAD_DIM = 64
B, N = 8, 1024
P = 128
KC = DIM // P          # 6 contraction chunks of 128 over channels
TC = N // P            # 8 token chunks of 128
HP = NUM_HEADS // 2    # 6 head pairs
VW = 128               # v columns per head: 64 v + 1 ones + 63 zero pad

F32 = mybir.dt.float32
F32R = mybir.dt.float32r
FP16 = mybir.dt.float16


def build_nc(with_qkv_bias: bool, with_proj_bias: bool):
    nc = bacc.Bacc("TRN2", target_bir_lowering=False, debug=False)

    xT_d = nc.dram_tensor("xT", [DIM, N], FP16, kind="ExternalInput")
    wqk_d = nc.dram_tensor("wqk", [DIM, 2 * DIM], FP16, kind="ExternalInput")
    wv_d = nc.dram_tensor("wv", [DIM, DIM], FP16, kind="ExternalInput")
    wproj_d = nc.dram_tensor("wproj", [DIM, DIM], FP16, kind="ExternalInput")
    bqk_d = nc.dram_tensor("bqk", [1, 2 * DIM], F32, kind="ExternalInput")
    bv_d = nc.dram_tensor("bv", [1, DIM], F32, kind="ExternalInput")
    bproj_d = nc.dram_tensor("bproj", [1, DIM], F32, kind="ExternalInput")
    ident_d = nc.dram_tensor("ident", [P, P], FP16, kind="ExternalInput")
    out_d = nc.dram_tensor("out", [N, DIM], FP16, kind="ExternalOutput")

    with tile.TileContext(nc) as tc:
        with (
            tc.tile_pool(name="consts", bufs=1) as consts,
            tc.tile_pool(name="inputs", bufs=1) as in_pool,
            tc.tile_pool(name="qk_sb", bufs=1) as qk_pool,
            tc.tile_pool(name="vaug_sb", bufs=1) as vaug_pool,
            tc.tile_pool(name="outT_sb", bufs=1) as outT_pool,
            tc.tile_pool(name="expT", bufs=10) as exp_pool,
            tc.tile_pool(name="avsb", bufs=2) as avsb_pool,
            tc.tile_pool(name="norm", bufs=4) as norm_pool,
            tc.tile_pool(name="rep", bufs=4) as rep_pool,
            tc.tile_pool(name="fin", bufs=4) as fin_pool,
            tc.tile_pool(name="partial", bufs=1) as partial_pool,
            tc.tile_pool(name="dramp", bufs=1, space="DRAM") as dram_pool,
            tc.tile_pool(name="ps_sc", bufs=2, space="PSUM") as ps_sc,
            tc.tile_pool(name="ps_av", bufs=2, space="PSUM") as ps_av,
        ):
            # ---- ACT exp table preload: a dummy exp so the ~2.7us
            # ACT_TABLE_LOAD happens during the input DMA wait ----
            warm_in = consts.tile([1, 16], F32)
            nc.vector.memset(warm_in[:], 0.0)
            warm_out = consts.tile([1, 16], FP16)
            nc.scalar.activation(
                warm_out[:], warm_in[:], mybir.ActivationFunctionType.Exp
            )
            ones_bc_f = consts.tile([1, 64], F32, name="ones_bc_f")
            nc.vector.memset(ones_bc_f[:], 1.0)
            ones_bc = consts.tile([1, 64], F32R, name="ones_bc")
            nc.vector.tensor_copy(ones_bc[:], ones_bc_f[:])
            # PE warmup: HAM un-throttles after ~3.4us of sustained matmul
            # activity; run junk matmuls from t~1us (const operands), then
            # junk paced by the arriving xT DMAs, so the real qkT chain
            # starts at the 2.4GHz clock instead of 1.2.
            warm_w = consts.tile([P, P], FP16, name="warm_w")
            nc.vector.memset(warm_w[:], 0.0)
            warm_x = consts.tile([P, 512], FP16, name="warm_x")
            nc.vector.memset(warm_x[:], 0.0)

            # ---- constants (bias path only; biases are zero here) ----
            if with_qkv_bias or with_proj_bias:
                ones_f32 = consts.tile([1, N], F32)
                nc.vector.memset(ones_f32[:], 1.0)
                ones_row = consts.tile([1, N], FP16)
                nc.vector.tensor_copy(ones_row[:], ones_f32[:])
            if with_qkv_bias:
                bqk_f32 = consts.tile([1, 2 * DIM], F32)
                nc.sync.dma_start(out=bqk_f32[:], in_=bqk_d[:])
                bqk_sb = consts.tile([1, 2 * DIM], FP16)
                nc.vector.tensor_copy(bqk_sb[:], bqk_f32[:])
                bv_f32 = consts.tile([1, DIM], F32)
                nc.sync.dma_start(out=bv_f32[:], in_=bv_d[:])
                bv_sb = consts.tile([1, DIM], FP16)
                nc.vector.tensor_copy(bv_sb[:], bv_f32[:])
            if with_proj_bias:
                bproj_f32 = consts.tile([1, DIM], F32)
                bproj_sb = consts.tile([1, DIM], FP16)

            # ---- input DMAs. wqk columns split so m-chunks 0 (q, cols
            # 0:128) and 6 (k, cols 768:896) land first -> the pair-0
            # scores chain unblocks ~4us earlier. ----
            xT = [in_pool.tile([P, N], FP16, name=f"xT{c}") for c in range(KC)]
            wqk = [
                in_pool.tile([P, 2 * DIM], FP16, name=f"wqk{c}") for c in range(KC)
            ]
            wv = [in_pool.tile([P, DIM], FP16, name=f"wv{c}") for c in range(KC)]
            wproj = [
                in_pool.tile([P, DIM], FP16, name=f"wproj{c}") for c in range(KC)
            ]
            ident = consts.tile([P, P], FP16, name="ident")
            nc.gpsimd.dma_start(out=ident[:], in_=ident_d[:])
            # xT on the Sync queue; the first-needed wqk column slices
            # (m-chunks 0 and 6) on the Scalar queue in parallel -- each
            # DMA costs ~650ns of descriptor generation on its queue
            # regardless of size, so one serial queue gated the first
            # exp by ~10us.
            for c in range(KC):
                sl = slice(c * P, (c + 1) * P)
                nc.sync.dma_start(out=xT[c][:], in_=xT_d[sl, :])
            for c in range(KC):
                sl = slice(c * P, (c + 1) * P)
                nc.scalar.dma_start(out=wqk[c][:, 0:P], in_=wqk_d[sl, 0:P])
            for c in range(KC):
                sl = slice(c * P, (c + 1) * P)
                nc.scalar.dma_start(
                    out=wqk[c][:, DIM : DIM + P], in_=wqk_d[sl, DIM : DIM + P]
                )
            for c in range(KC):
                sl = slice(c * P, (c + 1) * P)
                nc.sync.dma_start(out=wv[c][:], in_=wv_d[sl, :])
            for c in range(KC):
                sl = slice(c * P, (c + 1) * P)
                nc.sync.dma_start(out=wqk[c][:, P:DIM], in_=wqk_d[sl, P:DIM])
                nc.sync.dma_start(
                    out=wqk[c][:, DIM + P :], in_=wqk_d[sl, DIM + P :]
                )
            for c in range(KC):
                nc.sync.dma_start(
                    out=wproj[c][:], in_=wproj_d[c * P : (c + 1) * P, :]
                )
            if with_proj_bias:
                nc.sync.dma_start(out=bproj_f32[:], in_=bproj_d[:])
                nc.vector.tensor_copy(bproj_sb[:], bproj_f32[:])

            # ---- PE warmup: 12 const-gated + 2 per arriving xT chunk ----
            ps_warm = ps_sc.tile([P, 512], F32, name="ps_warm", tag="sc")
            n_warm = 12
            for i in range(n_warm):
                nc.tensor.matmul(
                    ps_warm[:], warm_w[:], warm_x[:],
                    start=(i == 0), stop=False,
                )
            for c in range(KC):
                for i in range(2):
                    nc.tensor.matmul(
                        ps_warm[:], warm_w[:], xT[c][:, 0:512],
                        start=False, stop=(c == KC - 1) and (i == 1),
                    )

            # ---- persistent tiles ----
            qkT = [
                qk_pool.tile([P, N], FP16, name=f"qkT{m}") for m in range(2 * KC)
            ]  # m 0-5: q chunk for pair m; 6-11: k chunk for pair m-6.
            #    head even in partitions 0:64, head odd in 64:128.
            v_aug = [
                vaug_pool.tile([P, NUM_HEADS * VW], FP16, name=f"vaug{t}")
                for t in range(TC)
            ]
            outT = [
                outT_pool.tile([P, N], FP16, name=f"outT{p}") for p in range(HP)
            ]
            partials = [
                partial_pool.tile([P, DIM], FP16, name=f"pjpart{t}") for t in range(TC)
            ]
            recip_d = dram_pool.tile([NUM_HEADS, N], F32)

            # ---- filler emitters: qkv/v/proj matmul groups the scheduler
            # slots into PE idle gaps while ACT runs exp ----
            def emit_qkT(m, q=None):
                qs = range(2) if q is None else [q]
                width = N if q is None else 512
                ps = ps_sc.tile([P, width], F32, name=f"ps_qk{m}_{qs[0]}", tag="sc")
                msl = slice(m * P, (m + 1) * P)
                for c in range(KC):
                    for qi, qq in enumerate(qs):
                        qsl = slice(qq * 512, (qq + 1) * 512)
                        osl = slice(qi * 512, (qi + 1) * 512)
                        nc.tensor.matmul(
                            ps[:, osl],
                            wqk[c][:, msl],
                            xT[c][:, qsl],
                            start=(c == 0),
                            stop=(c == KC - 1) and not with_qkv_bias,
                        )
                if with_qkv_bias:
                    for qi, qq in enumerate(qs):
                        qsl = slice(qq * 512, (qq + 1) * 512)
                        osl = slice(qi * 512, (qi + 1) * 512)
                        nc.tensor.matmul(
                            ps[:, osl],
                            bqk_sb[:, msl],
                            ones_row[:, qsl],
                            start=False,
                            stop=True,
                        )
                for qi, qq in enumerate(qs):
                    qsl = slice(qq * 512, (qq + 1) * 512)
                    osl = slice(qi * 512, (qi + 1) * 512)
                    nc.vector.tensor_copy(qkT[m][:, qsl], ps[:, osl])

            def emit_v(t):
                ps = ps_sc.tile([P, DIM], F32, name=f"ps_v{t}", tag="sc")
                tsl = slice(t * P, (t + 1) * P)
                for c in range(KC):
                    for nsl in (slice(0, 512), slice(512, DIM)):
                        nc.tensor.matmul(
                            ps[:, nsl],
                            xT[c][:, tsl],
                            wv[c][:, nsl],
                            start=(c == 0),
                            stop=(c == KC - 1) and not with_qkv_bias,
                        )
                if with_qkv_bias:
                    for nsl in (slice(0, 512), slice(512, DIM)):
                        nc.tensor.matmul(
                            ps[:, nsl],
                            ones_row[:, t * P : t * P + P],
                            bv_sb[:, nsl],
                            start=False,
                            stop=True,
                        )
                va3 = v_aug[t][:].rearrange("p (h e) -> p h e", e=VW)
                nc.vector.memset(va3[:, :, 64:65], 1.0)
                nc.vector.memset(va3[:, :, 65:VW], 0.0)
                nc.vector.tensor_copy(
                    va3[:, :, 0:64],
                    ps[:].rearrange("p (h d) -> p h d", d=HEAD_DIM),
                )

            # proj 3-way split: A = c 0..2 (outT pairs 0-2 exist after
            # pair 2), B = c 3..4 accumulated on top via DVE add, finish
            # = c 5 after the last pair's normalization.
            def emit_proj_a(t):
                ps = ps_sc.tile([P, DIM], F32, name=f"pja{t}", tag="sc")
                tsl = slice(t * P, (t + 1) * P)
                for c in range(3):
                    for nsl in (slice(0, 512), slice(512, DIM)):
                        nc.tensor.matmul(
                            ps[:, nsl],
                            outT[c][:, tsl],
                            wproj[c][:, nsl],
                            start=(c == 0),
                            stop=(c == 2),
                        )
                nc.vector.tensor_copy(partials[t][:], ps[:])

            def emit_proj_b(t):
                # c3 + c4 + running partial (re-injected via identity
                # matmul: cheaper than a serial DVE add on the PE-idle
                # epilogue path, and keeps partials in fp16)
                ps = ps_sc.tile([P, DIM], F32, name=f"pjb{t}", tag="sc")
                tsl = slice(t * P, (t + 1) * P)
                for c in (3, 4):
                    for nsl in (slice(0, 512), slice(512, DIM)):
                        nc.tensor.matmul(
                            ps[:, nsl],
                            outT[c][:, tsl],
                            wproj[c][:, nsl],
                            start=(c == 3),
                            stop=False,
                        )
                for nsl in (slice(0, 512), slice(512, DIM)):
                    nc.tensor.matmul(
                        ps[:, nsl], ident[:], partials[t][:, nsl],
                        start=False, stop=True,
                    )
                nc.vector.tensor_copy(partials[t][:], ps[:])

            def emit_proj_finish(t):
                ps = ps_sc.tile([P, DIM], F32, name=f"pjf{t}", tag="sc")
                tsl = slice(t * P, (t + 1) * P)
                # ident (partials) first: runs during the normalization
                # chain; the outT-dependent c5 matmuls close the group
                for nsl in (slice(0, 512), slice(512, DIM)):
                    nc.tensor.matmul(
                        ps[:, nsl], ident[:], partials[t][:, nsl],
                        start=True,
                        stop=False,
                    )
                for nsl in (slice(0, 512), slice(512, DIM)):
                    nc.tensor.matmul(
                        ps[:, nsl],
                        outT[KC - 1][:, tsl],
                        wproj[KC - 1][:, nsl],
                        start=False,
                        stop=not with_proj_bias,
                    )
                if with_proj_bias:
                    for nsl in (slice(0, 512), slice(512, DIM)):
                        nc.tensor.matmul(
                            ps[:, nsl],
                            ones_row[:, t * P : t * P + P],
                            bproj_sb[:, nsl],
                            start=False,
                            stop=True,
                        )
                fin = fin_pool.tile([P, DIM], FP16, name=f"fin{t}", tag="fin")
                nc.vector.tensor_copy(fin[:], ps[:])
                nc.gpsimd.dma_start(out=out_d[tsl, :], in_=fin[:])

            # Filler schedule, keyed (pair, kc-step). Legality: pair p
            # scores need qkT[p] fully and qkT[6+p] half0 by kc0 / half1
            # by kc4; v_aug[kc] is consumed at every pair's step kc.
            fillers = {p: {} for p in range(HP)}
            fillers[0] = {
                0: [(emit_v, (0,)), (emit_v, (1,))], 1: [(emit_v, (2,))], 2: [(emit_v, (3,))],
                3: [(emit_v, (4,))], 4: [(emit_v, (5,))],
                5: [(emit_v, (6,)), (emit_qkT, (1, 0))],
                6: [(emit_v, (7,)), (emit_qkT, (1, 1))],
                7: [(emit_qkT, (7, 0)), (emit_qkT, (7, 1))],
            }
            fillers[1] = {
                0: [(emit_qkT, (2, 0))], 2: [(emit_qkT, (2, 1))],
                4: [(emit_qkT, (8, 0))], 6: [(emit_qkT, (8, 1))],
            }
            fillers[2] = {
                0: [(emit_qkT, (3, 0))], 2: [(emit_qkT, (3, 1))],
                4: [(emit_qkT, (9, 0))], 6: [(emit_qkT, (9, 1))],
                1: [(emit_qkT, (4, 0))], 3: [(emit_qkT, (4, 1))],
                5: [(emit_qkT, (10, 0))],
                7: [(emit_qkT, (10, 1))],
            }
            fillers[3] = {
                1: [(emit_qkT, (5, 0))], 2: [(emit_qkT, (5, 1))],
                5: [(emit_qkT, (11, 0))], 7: [(emit_qkT, (11, 1))],
                3: [(emit_proj_a, (0,))], 4: [(emit_proj_a, (1,))],
                6: [(emit_proj_a, (2,))],
            }
            fillers[4] = {
                0: [(emit_proj_a, (3,))], 2: [(emit_proj_a, (4,))],
                4: [(emit_proj_a, (5,))], 5: [(emit_proj_a, (6,))],
                6: [(emit_proj_a, (7,))],
            }
            fillers[5] = {
                kc: [(emit_proj_b, (kc - 3,))] for kc in range(3, TC)
            }

            # ---- prologue: pair 0 operands (dense PE work during the
            # input DMA stream keeps the clock ramping) ----
            emit_qkT(0, 0)
            emit_qkT(6, 0)

            # ---- attention: 6 head pairs; even/odd scores matmuls hit
            # disjoint PE row-groups; one N=1024 exp per (kc, q-half)
            # covers both heads ----
            def norm_A(p, av_e, av_o):
                st = {"avsb": [], "recip": [], "rep": []}
                for i, av in ((0, av_e), (1, av_o)):
                    h = 2 * p + i
                    # ACT (fast PSUM port, idle at boundaries) copies av
                    # out of PSUM so the banks release in ~1.2us; sums
                    # row + reciprocal on DVE in parallel. reciprocal
                    # input must be a base-partition-0 [1,N] tile --
                    # feeding it offset-64 rows corrupted on HW.
                    av_sb = avsb_pool.tile([64, N], F32, name=f"avsb{h}", tag="avsb")
                    nc.scalar.copy(av_sb[:], av[0:64, :])
                    sums_t = norm_pool.tile([1, N], F32, name=f"sums{h}", tag="sums")
                    nc.vector.tensor_copy(sums_t[:], av[64:65, :])
                    recip_t = norm_pool.tile([1, N], F32, name=f"recip{h}", tag="recip")
                    nc.vector.reciprocal_approx_fast(out=recip_t[:], in_=sums_t[:])
                    recip_r = norm_pool.tile(
                        [1, N], F32R, name=f"recipr{h}", tag="recipr"
                    )
                    nc.vector.tensor_copy(recip_r[:], recip_t[:])
                    st["avsb"].append(av_sb)
                    st["recip"].append(recip_r)
                return st

            def norm_B(st):
                # partition-broadcast 1/sums on the PE: rep = ones^T @
                # recip (K=1 f32r matmul) -- replaces the DRAM round trip
                # whose DMA latency held up every pair boundary.
                for i in range(2):
                    rep = ps_av.tile([64, N], F32, name="rep_ps", tag="av")
                    for qsl in (slice(0, 512), slice(512, N)):
                        nc.tensor.matmul(
                            rep[:, qsl],
                            ones_bc[:],
                            st["recip"][i][0:1, qsl],
                            start=True, stop=True,
                        )
                    st["rep"].append(rep)

            def norm_C(p, st, split=False):
                for i in range(2):
                    hrow = slice(i * 64, (i + 1) * 64)
                    halves = (
                        (slice(0, 512), slice(512, N)) if split else (slice(0, N),)
                    )
                    for half in halves:
                        nc.vector.tensor_tensor(
                            out=outT[p][hrow, half],
                            in0=st["avsb"][i][:, half],
                            in1=st["rep"][i][:, half],
                            op=mybir.AluOpType.mult,
                        )

            AVD = 3  # av trails scores/exp by 3 kc steps so the previous
            # pair's staged normalization (next pair kc0/1/2) is emitted
            # before this pair's first av write (PSUM WAR ordering)
            pending = None
            for p in range(HP):
                qT = qkT[p]
                kT = qkT[HP + p]
                av_e = ps_av.tile([P, N], F32, name=f"av{2 * p}", tag="av")
                av_o = ps_av.tile([P, N], F32, name=f"av{2 * p + 1}", tag="av")

                def emit_av(kc, eTs, av_e=av_e, av_o=av_o, p=p):
                    for qh in range(2):
                        qsl = slice(qh * 512, (qh + 1) * 512)
                        nc.tensor.matmul(
                            av_e[:, qsl],
                            v_aug[kc][:, (2 * p) * VW : (2 * p) * VW + VW],
                            eTs[qh][:, 0:512],
                            start=(kc == 0), stop=(kc == TC - 1),
                        )
                        nc.tensor.matmul(
                            av_o[:, qsl],
                            v_aug[kc][:, (2 * p + 1) * VW : (2 * p + 1) * VW + VW],
                            eTs[qh][:, 512:1024],
                            start=(kc == 0), stop=(kc == TC - 1),
                        )

                eT_hist = {}
                norm_st = None
                for kc in range(TC):
                    ksl = slice(kc * P, (kc + 1) * P)
                    eTs = []
                    for qh in range(2):
                        qsl = slice(qh * 512, (qh + 1) * 512)
                        sc = ps_sc.tile(
                            [P, N], F32, name=f"sc{p}_{kc}_{qh}", tag="sc"
                        )
                        nc.tensor.matmul(
                            sc[:, 0:512], kT[0:64, ksl], qT[0:64, qsl],
                            start=True, stop=True,
                        )
                        nc.tensor.matmul(
                            sc[:, 512:1024], kT[64:128, ksl], qT[64:128, qsl],
                            start=True, stop=True,
                        )
                        eT = exp_pool.tile(
                            [P, N], FP16, name=f"e{p}_{kc}_{qh}", tag="e"
                        )
                        nc.scalar.activation(
                            eT[:], sc[:], mybir.ActivationFunctionType.Exp
                        )
                        eTs.append(eT)
                        if p == 0 and kc == 0 and qh == 0:
                            # second q-halves of the pair-0 operands land
                            # here so the first exp isn't gated on them
                            emit_qkT(0, 1)
                            emit_qkT(6, 1)
                    eT_hist[kc] = eTs
                    if kc >= AVD:
                        emit_av(kc - AVD, eT_hist.pop(kc - AVD))
                    if pending is not None:
                        if kc == 0:
                            norm_st = norm_A(*pending)
                        elif kc == 1:
                            norm_B(norm_st)
                        elif kc == 2:
                            norm_C(pending[0], norm_st)
                    for fn, args in fillers[p].get(kc, []):
                        fn(*args)
                for kc in range(TC - AVD, TC):
                    emit_av(kc, eT_hist.pop(kc))
                pending = (p, av_e, av_o)

            # ---- tail: last pair normalize staged between the proj Bs
            # (keeps the PE HAM-warm through the chain) ----
            st = norm_A(*pending)
            emit_proj_b(5)
            norm_B(st)
            emit_proj_b(6)
            norm_C(pending[0], st, split=True)
            emit_proj_b(7)
            for t in range(TC):
                emit_proj_finish(t)

    nc.compile()
    return nc


def prep_in_maps(inputs):
    x = np.asarray(inputs["x"], dtype=np.float32)
    qkv_w = np.asarray(inputs["qkv_w"], dtype=np.float32)
    qkv_b = np.asarray(inputs["qkv_b"], dtype=np.float32)
    proj_w = np.asarray(inputs["proj_w"], dtype=np.float32)
    proj_b = np.asarray(inputs["proj_b"], dtype=np.float32)
    # context is unused by the reference layer.

    scale = HEAD_DIM ** -0.5
    wqk = qkv_w[:, : 2 * DIM].copy()
    wqk[:, :DIM] *= scale
    bqk = qkv_b[: 2 * DIM].copy()
    bqk[:DIM] *= scale

    base = {
        "ident": np.eye(128, dtype=np.float16),
        "wqk": wqk.astype(np.float16),
        "wv": np.ascontiguousarray(qkv_w[:, 2 * DIM :]).astype(np.float16),
        "wproj": proj_w.astype(np.float16),
        "bqk": bqk.reshape(1, -1).astype(np.float32),
        "bv": qkv_b[2 * DIM :].reshape(1, -1).astype(np.float32),
        "bproj": proj_b.reshape(1, -1).astype(np.float32),
    }
    in_maps = [
        {**base, "xT": np.ascontiguousarray(x[b].T).astype(np.float16)}
        for b in range(B)
    ]
    with_qkv_bias = bool(np.any(qkv_b))
    with_proj_bias = bool(np.any(proj_b))
    return in_maps, with_qkv_bias, with_proj_bias


_NC_CACHE = {}


def kernel(**inputs) -> np.ndarray:
    in_maps, with_qkv_bias, with_proj_bias = prep_in_maps(inputs)
    key = (with_qkv_bias, with_proj_bias)
    if key not in _NC_CACHE:
        _NC_CACHE[key] = build_nc(*key)
    nc = _NC_CACHE[key]
    res = run_bass_kernel_spmd(nc, in_maps, list(range(B)))
    out = np.stack([res.results[b]["out"] for b in range(B)], axis=0)
    return out.astype(np.float32)


# revision 28
# speedup vs baseline: 1.0421x; 1.0421x over previous
"""CrossModalAttention kernel for 8x TRN2 NeuronCores (batch data-parallel).

Reference computation (per batch element b, context input is unused):
    qkv = x @ qkv_w + qkv_b            # [N, 3C]
    q, k, v = split(qkv)               # heads H=12, d=64
    attn = softmax(q*scale @ k^T)      # per head, N=1024
    out = (attn @ v) @ proj_w + proj_b # [N, C]

Strategy per core (one batch element each). v5 -- all-fp16 matmuls
(fp8/DoubleRow measured 7e-2 rel err: ~3% per-element quantization does
NOT average out in random-sign contractions), head-pair loop structure,
stall elimination:

  - Head-PAIR scores: head 2p lives in partitions 0:64 and head 2p+1 in
    64:128 of the qkT chunks, so the two scores matmuls (K=64) target
    disjoint PE row-groups via auto tile_position and partially overlap.
    One [128,1024] PSUM tile per (kc, q-half) = even|odd halves, one
    N=1024 exp covers both heads (minimizes ACT instruction count; ACT
    exp is the second-busiest engine at ~110us).
  - v_aug padded to 128 cols/head (64 v dims + ones + 63 zeros): full
    128-col stationary operand re-enables fast weight load; M=65 was
    measured +40%/matmul from serialized LDWEIGHTS. PSUM cost unchanged
    (rows 65:128 of the av banks were dead anyway).
  - Normalization: the whole av [65,1024] is copied PSUM->SBUF first,
    releasing the PSUM bank in ~0.8us instead of holding it through the
    reciprocal + DRAM-broadcast chain (~5us); this killed the ~5us
    per-pair-boundary ACT stalls seen in the v4 trace. Then 1/sums via
    DVE fast reciprocal, partition-broadcast via DRAM round trip, DVE
    multiply into fp16 outT.
  - qkv weight DMAs split so the first columns (m-chunks 0 and 6) land
    first: the first scores matmul unblocks ~4us earlier.
  - proj split 3 ways: A = c 0..2 (after pair 2), B = c 3..4 (+DVE add,
    during pair 5), finish = c5 after the last normalization.
  PSUM (8 banks): sc ring 2x[128,1024] = 4 (also carries qkv/v/proj
  filler groups), av_e+av_o 2x[128,1024] = 4.
"""
import numpy as np

import concourse.bass as bass
import concourse.tile as tile
from concourse import bacc, mybir
from concourse.bass_utils import run_bass_kernel_spmd

DIM = 768
NUM_HEADS = 12
HEAD_DIM = 64
B, N = 8, 1024
P = 128
KC = DIM // P          # 6 contraction chunks of 128 over channels
TC = N // P            # 8 token chunks of 128
HP = NUM_HEADS // 2    # 6 head pairs
VW = 128               # v columns per head: 64 v + 1 ones + 63 zero pad

F32 = mybir.dt.float32
F32R = mybir.dt.float32r
FP16 = mybir.dt.float16


def build_nc(with_qkv_bias: bool, with_proj_bias: bool):
    nc = bacc.Bacc("TRN2", target_bir_lowering=False, debug=False)

    xT_d = nc.dram_tensor("xT", [DIM, N], FP16, kind="ExternalInput")
    wqk_d = nc.dram_tensor("wqk", [DIM, 2 * DIM], FP16, kind="ExternalInput")
    wv_d = nc.dram_tensor("wv", [DIM, DIM], FP16, kind="ExternalInput")
    wproj_d = nc.dram_tensor("wproj", [DIM, DIM], FP16, kind="ExternalInput")
    bqk_d = nc.dram_tensor("bqk", [1, 2 * DIM], F32, kind="ExternalInput")
    bv_d = nc.dram_tensor("bv", [1, DIM], F32, kind="ExternalInput")
    bproj_d = nc.dram_tensor("bproj", [1, DIM], F32, kind="ExternalInput")
    ident_d = nc.dram_tensor("ident", [P, P], FP16, kind="ExternalInput")
    out_d = nc.dram_tensor("out", [N, DIM], FP16, kind="ExternalOutput")

    with tile.TileContext(nc) as tc:
        with (
            tc.tile_pool(name="consts", bufs=1) as consts,
            tc.tile_pool(name="inputs", bufs=1) as in_pool,
            tc.tile_pool(name="qk_sb", bufs=1) as qk_pool,
            tc.tile_pool(name="vaug_sb", bufs=1) as vaug_pool,
            tc.tile_pool(name="outT_sb", bufs=1) as outT_pool,
            tc.tile_pool(name="expT", bufs=11) as exp_pool,
            tc.tile_pool(name="avsb", bufs=2) as avsb_pool,
            tc.tile_pool(name="norm", bufs=4) as norm_pool,
            tc.tile_pool(name="fin", bufs=4) as fin_pool,
            tc.tile_pool(name="partial", bufs=1) as partial_pool,
            tc.tile_pool(name="dramp", bufs=1, space="DRAM") as dram_pool,
            tc.tile_pool(name="ps_sc", bufs=2, space="PSUM") as ps_sc,
            tc.tile_pool(name="ps_av", bufs=2, space="PSUM") as ps_av,
        ):
            # ---- ACT exp table preload: a dummy exp so the ~2.7us
            # ACT_TABLE_LOAD happens during the input DMA wait ----
            warm_in = consts.tile([1, 16], F32)
            nc.vector.memset(warm_in[:], 0.0)
            warm_out = consts.tile([1, 16], FP16)
            nc.scalar.activation(
                warm_out[:], warm_in[:], mybir.ActivationFunctionType.Exp
            )
            ones_bc_f = consts.tile([1, 64], F32, name="ones_bc_f")
            nc.vector.memset(ones_bc_f[:], 1.0)
            ones_bc = consts.tile([1, 64], F32R, name="ones_bc")
            nc.vector.tensor_copy(ones_bc[:], ones_bc_f[:])
            # PE warmup: HAM un-throttles after ~3.4us of sustained matmul
            # activity; run junk matmuls from t~1us (const operands), then
            # junk paced by the arriving xT DMAs, so the real qkT chain
            # starts at the 2.4GHz clock instead of 1.2.
            warm_w = consts.tile([P, P], FP16, name="warm_w")
            nc.vector.memset(warm_w[:], 0.0)
            warm_x = consts.tile([P, 512], FP16, name="warm_x")
            nc.vector.memset(warm_x[:], 0.0)

            # ---- constants (bias path only; biases are zero here) ----
            if with_qkv_bias or with_proj_bias:
                ones_f32 = consts.tile([1, N], F32)
                nc.vector.memset(ones_f32[:], 1.0)
                ones_row = consts.tile([1, N], FP16)
                nc.vector.tensor_copy(ones_row[:], ones_f32[:])
            if with_qkv_bias:
                bqk_f32 = consts.tile([1, 2 * DIM], F32)
                nc.sync.dma_start(out=bqk_f32[:], in_=bqk_d[:])
                bqk_sb = consts.tile([1, 2 * DIM], FP16)
                nc.vector.tensor_copy(bqk_sb[:], bqk_f32[:])
                bv_f32 = consts.tile([1, DIM], F32)
                nc.sync.dma_start(out=bv_f32[:], in_=bv_d[:])
                bv_sb = consts.tile([1, DIM], FP16)
                nc.vector.tensor_copy(bv_sb[:], bv_f32[:])
            if with_proj_bias:
                bproj_f32 = consts.tile([1, DIM], F32)
                bproj_sb = consts.tile([1, DIM], FP16)

            # ---- input DMAs. wqk columns split so m-chunks 0 (q, cols
            # 0:128) and 6 (k, cols 768:896) land first -> the pair-0
            # scores chain unblocks ~4us earlier. ----
            xT = [in_pool.tile([P, N], FP16, name=f"xT{c}") for c in range(KC)]
            wqk = [
                in_pool.tile([P, 2 * DIM], FP16, name=f"wqk{c}") for c in range(KC)
            ]
            wv = [in_pool.tile([P, DIM], FP16, name=f"wv{c}") for c in range(KC)]
            wproj = [
                in_pool.tile([P, DIM], FP16, name=f"wproj{c}") for c in range(KC)
            ]
            ident = consts.tile([P, P], FP16, name="ident")
            nc.gpsimd.dma_start(out=ident[:], in_=ident_d[:])
            # xT on the Sync queue; the first-needed wqk column slices
            # (m-chunks 0 and 6) on the Scalar queue in parallel -- each
            # DMA costs ~650ns of descriptor generation on its queue
            # regardless of size, so one serial queue gated the first
            # exp by ~10us.
            for c in range(KC):
                sl = slice(c * P, (c + 1) * P)
                nc.sync.dma_start(out=xT[c][:], in_=xT_d[sl, :])
            for c in range(KC):
                sl = slice(c * P, (c + 1) * P)
                nc.scalar.dma_start(out=wqk[c][:, 0:P], in_=wqk_d[sl, 0:P])
            for c in range(KC):
                sl = slice(c * P, (c + 1) * P)
                nc.scalar.dma_start(
                    out=wqk[c][:, DIM : DIM + P], in_=wqk_d[sl, DIM : DIM + P]
                )
            for c in range(KC):
                sl = slice(c * P, (c + 1) * P)
                nc.sync.dma_start(out=wv[c][:], in_=wv_d[sl, :])
            for c in range(KC):
                sl = slice(c * P, (c + 1) * P)
                nc.sync.dma_start(out=wqk[c][:, P:DIM], in_=wqk_d[sl, P:DIM])
                nc.sync.dma_start(
                    out=wqk[c][:, DIM + P :], in_=wqk_d[sl, DIM + P :]
                )
            for c in range(KC):
                nc.sync.dma_start(
                    out=wproj[c][:], in_=wproj_d[c * P : (c + 1) * P, :]
                )
            if with_proj_bias:
                nc.sync.dma_start(out=bproj_f32[:], in_=bproj_d[:])
                nc.vector.tensor_copy(bproj_sb[:], bproj_f32[:])

            # ---- PE warmup: 12 const-gated + 2 per arriving xT chunk ----
            ps_warm = ps_sc.tile([P, 512], F32, name="ps_warm", tag="sc")
            n_warm = 12
            for i in range(n_warm):
                nc.tensor.matmul(
                    ps_warm[:], warm_w[:], warm_x[:],
                    start=(i == 0), stop=False,
                )
            for c in range(KC):
                for i in range(2):
                    nc.tensor.matmul(
                        ps_warm[:], warm_w[:], xT[c][:, 0:512],
                        start=False, stop=(c == KC - 1) and (i == 1),
                    )

            # ---- persistent tiles ----
            qkT = [
                qk_pool.tile([P, N], FP16, name=f"qkT{m}") for m in range(2 * KC)
            ]  # m 0-5: q chunk for pair m; 6-11: k chunk for pair m-6.
            #    head even in partitions 0:64, head odd in 64:128.
            v_aug = [
                vaug_pool.tile([P, NUM_HEADS * VW], FP16, name=f"vaug{t}")
                for t in range(TC)
            ]
            outT = [
                outT_pool.tile([P, N], FP16, name=f"outT{p}") for p in range(HP)
            ]
            partials = [
                partial_pool.tile([P, DIM], FP16, name=f"pjpart{t}") for t in range(TC)
            ]
            recip_d = dram_pool.tile([NUM_HEADS, N], F32)

            # ---- filler emitters: qkv/v/proj matmul groups the scheduler
            # slots into PE idle gaps while ACT runs exp ----
            def emit_qkT(m, q=None):
                qs = range(2) if q is None else [q]
                width = N if q is None else 512
                ps = ps_sc.tile([P, width], F32, name=f"ps_qk{m}_{qs[0]}", tag="sc")
                msl = slice(m * P, (m + 1) * P)
                for c in range(KC):
                    for qi, qq in enumerate(qs):
                        qsl = slice(qq * 512, (qq + 1) * 512)
                        osl = slice(qi * 512, (qi + 1) * 512)
                        nc.tensor.matmul(
                            ps[:, osl],
                            wqk[c][:, msl],
                            xT[c][:, qsl],
                            start=(c == 0),
                            stop=(c == KC - 1) and not with_qkv_bias,
                        )
                if with_qkv_bias:
                    for qi, qq in enumerate(qs):
                        qsl = slice(qq * 512, (qq + 1) * 512)
                        osl = slice(qi * 512, (qi + 1) * 512)
                        nc.tensor.matmul(
                            ps[:, osl],
                            bqk_sb[:, msl],
                            ones_row[:, qsl],
                            start=False,
                            stop=True,
                        )
                for qi, qq in enumerate(qs):
                    qsl = slice(qq * 512, (qq + 1) * 512)
                    osl = slice(qi * 512, (qi + 1) * 512)
                    nc.vector.tensor_copy(qkT[m][:, qsl], ps[:, osl])

            def emit_v(t):
                ps = ps_sc.tile([P, DIM], F32, name=f"ps_v{t}", tag="sc")
                tsl = slice(t * P, (t + 1) * P)
                for c in range(KC):
                    for nsl in (slice(0, 512), slice(512, DIM)):
                        nc.tensor.matmul(
                            ps[:, nsl],
                            xT[c][:, tsl],
                            wv[c][:, nsl],
                            start=(c == 0),
                            stop=(c == KC - 1) and not with_qkv_bias,
                        )
                if with_qkv_bias:
                    for nsl in (slice(0, 512), slice(512, DIM)):
                        nc.tensor.matmul(
                            ps[:, nsl],
                            ones_row[:, t * P : t * P + P],
                            bv_sb[:, nsl],
                            start=False,
                            stop=True,
                        )
                va3 = v_aug[t][:].rearrange("p (h e) -> p h e", e=VW)
                nc.vector.memset(va3[:, :, 64:65], 1.0)
                nc.vector.memset(va3[:, :, 65:VW], 0.0)
                nc.vector.tensor_copy(
                    va3[:, :, 0:64],
                    ps[:].rearrange("p (h d) -> p h d", d=HEAD_DIM),
                )

            # proj 3-way split: A = c 0..2 (outT pairs 0-2 exist after
            # pair 2), B = c 3..4 accumulated on top via DVE add, finish
            # = c 5 after the last pair's normalization.
            def emit_proj_a(t):
                ps = ps_sc.tile([P, DIM], F32, name=f"pja{t}", tag="sc")
                tsl = slice(t * P, (t + 1) * P)
                for c in range(3):
                    for nsl in (slice(0, 512), slice(512, DIM)):
                        nc.tensor.matmul(
                            ps[:, nsl],
                            outT[c][:, tsl],
                            wproj[c][:, nsl],
                            start=(c == 0),
                            stop=(c == 2),
                        )
                nc.vector.tensor_copy(partials[t][:], ps[:])

            def emit_proj_b(t):
                # c3 + c4 + running partial (re-injected via identity
                # matmul: cheaper than a serial DVE add on the PE-idle
                # epilogue path, and keeps partials in fp16)
                ps = ps_sc.tile([P, DIM], F32, name=f"pjb{t}", tag="sc")
                tsl = slice(t * P, (t + 1) * P)
                for c in (3, 4):
                    for nsl in (slice(0, 512), slice(512, DIM)):
                        nc.tensor.matmul(
                            ps[:, nsl],
                            outT[c][:, tsl],
                            wproj[c][:, nsl],
                            start=(c == 3),
                            stop=False,
                        )
                for nsl in (slice(0, 512), slice(512, DIM)):
                    nc.tensor.matmul(
                        ps[:, nsl], ident[:], partials[t][:, nsl],
                        start=False, stop=True,
                    )
                nc.vector.tensor_copy(partials[t][:], ps[:])

            def emit_proj_finish(t):
                ps = ps_sc.tile([P, DIM], F32, name=f"pjf{t}", tag="sc")
                tsl = slice(t * P, (t + 1) * P)
                # ident (partials) first: runs during the normalization
                # chain; the outT-dependent c5 matmuls close the group
                for nsl in (slice(0, 512), slice(512, DIM)):
                    nc.tensor.matmul(
                        ps[:, nsl], ident[:], partials[t][:, nsl],
                        start=True,
                        stop=False,
                    )
                for nsl in (slice(0, 512), slice(512, DIM)):
                    nc.tensor.matmul(
                        ps[:, nsl],
                        outT[KC - 1][:, tsl],
                        wproj[KC - 1][:, nsl],
                        start=False,
                        stop=not with_proj_bias,
                    )
                if with_proj_bias:
                    for nsl in (slice(0, 512), slice(512, DIM)):
                        nc.tensor.matmul(
                            ps[:, nsl],
                            ones_row[:, t * P : t * P + P],
                            bproj_sb[:, nsl],
                            start=False,
                            stop=True,
                        )
                fin = fin_pool.tile([P, DIM], FP16, name=f"fin{t}", tag="fin")
                nc.vector.tensor_copy(fin[:], ps[:])
                nc.gpsimd.dma_start(out=out_d[tsl, :], in_=fin[:])

            # Filler schedule, keyed (pair, kc-step). Legality: pair p
            # scores need qkT[p] fully and qkT[6+p] half0 by kc0 / half1
            # by kc4; v_aug[kc] is consumed at every pair's step kc.
            fillers = {p: {} for p in range(HP)}
            fillers[0] = {
                0: [(emit_v, (0,)), (emit_v, (1,))], 1: [(emit_v, (2,))], 2: [(emit_v, (3,))],
                3: [(emit_v, (4,))], 4: [(emit_v, (5,))],
                5: [(emit_v, (6,)), (emit_qkT, (1, 0))],
                6: [(emit_v, (7,)), (emit_qkT, (1, 1))],
                7: [(emit_qkT, (7, 0)), (emit_qkT, (7, 1))],
            }
            fillers[1] = {
                0: [(emit_qkT, (2, 0))], 2: [(emit_qkT, (2, 1))],
                4: [(emit_qkT, (8, 0))], 6: [(emit_qkT, (8, 1))],
            }
            fillers[2] = {
                0: [(emit_qkT, (3, 0))], 2: [(emit_qkT, (3, 1))],
                4: [(emit_qkT, (9, 0))], 6: [(emit_qkT, (9, 1))],
                1: [(emit_qkT, (4, 0))], 3: [(emit_qkT, (4, 1))],
                5: [(emit_qkT, (10, 0))],
                7: [(emit_qkT, (10, 1))],
            }
            fillers[3] = {
                1: [(emit_qkT, (5, 0))], 2: [(emit_qkT, (5, 1))],
                5: [(emit_qkT, (11, 0))], 7: [(emit_qkT, (11, 1))],
                3: [(emit_proj_a, (0,))], 4: [(emit_proj_a, (1,))],
                6: [(emit_proj_a, (2,))],
            }
            fillers[4] = {
                0: [(emit_proj_a, (3,))], 2: [(emit_proj_a, (4,))],
                4: [(emit_proj_a, (5,))], 5: [(emit_proj_a, (6,))],
                6: [(emit_proj_a, (7,))],
            }
            fillers[5] = {
                kc: [(emit_proj_b, (kc - 3,))] for kc in range(3, TC)
            }

            # ---- prologue: pair 0 operands (dense PE work during the
            # input DMA stream keeps the clock ramping) ----
            emit_qkT(0, 0)
            emit_qkT(6, 0)

            # ---- attention: 6 head pairs; even/odd scores matmuls hit
            # disjoint PE row-groups; one N=1024 exp per (kc, q-half)
            # covers both heads ----
            def norm_A(p, av_e, av_o):
                st = {"avsb": [], "recip": [], "rep": []}
                for i, av in ((0, av_e), (1, av_o)):
                    h = 2 * p + i
                    # ACT (fast PSUM port, idle at boundaries) copies av
                    # out of PSUM so the banks release in ~1.2us; sums
                    # row + reciprocal on DVE in parallel. reciprocal
                    # input must be a base-partition-0 [1,N] tile --
                    # feeding it offset-64 rows corrupted on HW.
                    av_sb = avsb_pool.tile([64, N], F32, name=f"avsb{h}", tag="avsb")
                    nc.scalar.copy(av_sb[:], av[0:64, :])
                    sums_t = norm_pool.tile([1, N], F32, name=f"sums{h}", tag="sums")
                    nc.vector.tensor_copy(sums_t[:], av[64:65, :])
                    recip_t = norm_pool.tile([1, N], F32, name=f"recip{h}", tag="recip")
                    nc.vector.reciprocal_approx_fast(out=recip_t[:], in_=sums_t[:])
                    recip_r = norm_pool.tile(
                        [1, N], F32R, name=f"recipr{h}", tag="recipr"
                    )
                    nc.vector.tensor_copy(recip_r[:], recip_t[:])
                    st["avsb"].append(av_sb)
                    st["recip"].append(recip_r)
                return st

            def norm_B(st):
                # partition-broadcast 1/sums on the PE: rep = ones^T @
                # recip (K=1 f32r matmul) -- replaces the DRAM round trip
                # whose DMA latency held up every pair boundary.
                for i in range(2):
                    rep = ps_av.tile([64, N], F32, name="rep_ps", tag="av")
                    for qsl in (slice(0, 512), slice(512, N)):
                        nc.tensor.matmul(
                            rep[:, qsl],
                            ones_bc[:],
                            st["recip"][i][0:1, qsl],
                            start=True, stop=True,
                        )
                    st["rep"].append(rep)

            def norm_C(p, st, split=False):
                for i in range(2):
                    hrow = slice(i * 64, (i + 1) * 64)
                    halves = (
                        (slice(0, 512), slice(512, N)) if split else (slice(0, N),)
                    )
                    for half in halves:
                        nc.vector.tensor_tensor(
                            out=outT[p][hrow, half],
                            in0=st["avsb"][i][:, half],
                            in1=st["rep"][i][:, half],
                            op=mybir.AluOpType.mult,
                        )

            AVD = 3  # av trails scores/exp by 3 kc steps so the previous
            # pair's staged normalization (next pair kc0/1/2) is emitted
            # before this pair's first av write (PSUM WAR ordering)
            pending = None
            for p in range(HP):
                qT = qkT[p]
                kT = qkT[HP + p]
                av_e = ps_av.tile([P, N], F32, name=f"av{2 * p}", tag="av")
                av_o = ps_av.tile([P, N], F32, name=f"av{2 * p + 1}", tag="av")

                def emit_av(kc, eTs, av_e=av_e, av_o=av_o, p=p):
                    for qh in range(2):
                        qsl = slice(qh * 512, (qh + 1) * 512)
                        nc.tensor.matmul(
                            av_e[:, qsl],
                            v_aug[kc][:, (2 * p) * VW : (2 * p) * VW + VW],
                            eTs[qh][:, 0:512],
                            start=(kc == 0), stop=(kc == TC - 1),
                        )
                        nc.tensor.matmul(
                            av_o[:, qsl],
                            v_aug[kc][:, (2 * p + 1) * VW : (2 * p + 1) * VW + VW],
                            eTs[qh][:, 512:1024],
                            start=(kc == 0), stop=(kc == TC - 1),
                        )

                eT_hist = {}
                norm_st = None
                for kc in range(TC):
                    ksl = slice(kc * P, (kc + 1) * P)
                    eTs = []
                    for qh in range(2):
                        qsl = slice(qh * 512, (qh + 1) * 512)
                        sc = ps_sc.tile(
                            [P, N], F32, name=f"sc{p}_{kc}_{qh}", tag="sc"
                        )
                        nc.tensor.matmul(
                            sc[:, 0:512], kT[0:64, ksl], qT[0:64, qsl],
                            start=True, stop=True,
                        )
                        nc.tensor.matmul(
                            sc[:, 512:1024], kT[64:128, ksl], qT[64:128, qsl],
                            start=True, stop=True,
                        )
                        eT = exp_pool.tile(
                            [P, N], FP16, name=f"e{p}_{kc}_{qh}", tag="e"
                        )
                        nc.scalar.activation(
                            eT[:], sc[:], mybir.ActivationFunctionType.Exp
                        )
                        eTs.append(eT)
                        if p == 0 and kc == 0 and qh == 0:
                            # second q-halves of the pair-0 operands land
                            # here so the first exp isn't gated on them
                            emit_qkT(0, 1)
                            emit_qkT(6, 1)
                    eT_hist[kc] = eTs
                    if kc >= AVD:
                        emit_av(kc - AVD, eT_hist.pop(kc - AVD))
                    if pending is not None:
                        if kc == 0:
                            norm_st = norm_A(*pending)
                        elif kc == 1:
                            norm_B(norm_st)
                        elif kc == 2:
                            norm_C(pending[0], norm_st)
                    for fn, args in fillers[p].get(kc, []):
                        fn(*args)
                for kc in range(TC - AVD, TC):
                    emit_av(kc, eT_hist.pop(kc))
                pending = (p, av_e, av_o)

            # ---- tail: last pair normalize staged between the proj Bs
            # (keeps the PE HAM-warm through the chain) ----
            st = norm_A(*pending)
            emit_proj_b(5)
            norm_B(st)
            emit_proj_b(6)
            norm_C(pending[0], st, split=True)
            emit_proj_b(7)
            for t in range(TC):
                emit_proj_finish(t)

    nc.compile()
    return nc


def prep_in_maps(inputs):
    x = np.asarray(inputs["x"], dtype=np.float32)
    qkv_w = np.asarray(inputs["qkv_w"], dtype=np.float32)
    qkv_b = np.asarray(inputs["qkv_b"], dtype=np.float32)
    proj_w = np.asarray(inputs["proj_w"], dtype=np.float32)
    proj_b = np.asarray(inputs["proj_b"], dtype=np.float32)
    # context is unused by the reference layer.

    scale = HEAD_DIM ** -0.5
    wqk = qkv_w[:, : 2 * DIM].copy()
    wqk[:, :DIM] *= scale
    bqk = qkv_b[: 2 * DIM].copy()
    bqk[:DIM] *= scale

    base = {
        "ident": np.eye(128, dtype=np.float16),
        "wqk": wqk.astype(np.float16),
        "wv": np.ascontiguousarray(qkv_w[:, 2 * DIM :]).astype(np.float16),
        "wproj": proj_w.astype(np.float16),
        "bqk": bqk.reshape(1, -1).astype(np.float32),
        "bv": qkv_b[2 * DIM :].reshape(1, -1).astype(np.float32),
        "bproj": proj_b.reshape(1, -1).astype(np.float32),
    }
    in_maps = [
        {**base, "xT": np.ascontiguousarray(x[b].T).astype(np.float16)}
        for b in range(B)
    ]
    with_qkv_bias = bool(np.any(qkv_b))
    with_proj_bias = bool(np.any(proj_b))
    return in_maps, with_qkv_bias, with_proj_bias


_NC_CACHE = {}


def kernel(**inputs) -> np.ndarray:
    in_maps, with_qkv_bias, with_proj_bias = prep_in_maps(inputs)
    key = (with_qkv_bias, with_proj_bias)
    if key not in _NC_CACHE:
        _NC_CACHE[key] = build_nc(*key)
    nc = _NC_CACHE[key]
    res = run_bass_kernel_spmd(nc, in_maps, list(range(B)))
    out = np.stack([res.results[b]["out"] for b in range(B)], axis=0)
    return out.astype(np.float32)
